# revision 12
# baseline (speedup 1.0000x reference)
"""Single-head causal attention (B=8, S=2048, E=2048, D=128) on 8 trn2 cores.

Sharding: data-parallel over batch — one batch element per NeuronCore.

Host marshaling per core: xT = x[b].T and WT = W.T, both cast to bf16
(the kernel computes in bf16 with f32 accumulation; casting during input
marshaling is numerically identical to casting on device).

Per-core dataflow (bf16 matmuls, f32 PSUM accumulation):
  - projections produce qT/kT/vT in [D, S] layout; bias added during the
    VectorE PSUM->SBUF evacuation (per-partition scalar add)
  - vT is re-transposed on the PE into natural [S, D] blocks, augmented
    with a ones column (col 128): the AV matmul then yields the softmax
    denominator for free as output column 128
  - scoresT[k, q] per k-block j: single matmul (K=D=128), exact causal
    trim of the q range; diagonal 128-block masked by adding -1e30;
    ScalarE computes exp(scale*s) straight out of PSUM into bf16 probsT
  - AV per q-block i accumulates probsT_j.T @ v_aug_j over j<=i in PSUM;
    VectorE takes 1/denominator and applies it during the final
    evacuation; DMA stores the natural-layout [128, 128] f32 result
"""

import math

import numpy as np

B = 8
S = 2048
E = 2048
D = 128
P = 128
NE = E // P  # 16 contraction chunks
NS = S // P  # 16 sequence blocks
ST = 512  # s-tile width for projections / score chunks
NST = S // ST  # 4
VW = D + 1  # v block width incl. ones column
SCALE = 1.0 / math.sqrt(S)
NEG = -1.0e30

_PROGRAMS = {}

# which phases to emit (for microbenchmarking): subset of
# {"proj", "vtrans", "scores", "av", "store"}
PHASES = frozenset({"proj", "vtrans", "scores", "av", "store"})

# projection matmul precision: "fp8" (DoubleRow, 2x PE) or "bf16"
PROJ_DTYPE = "bf16"
W_SCALE = 256.0  # host pre-scale of W before fp8 quantization


def build_program(iters=1):
    global _PROGRAMS
    key = (iters, PHASES, PROJ_DTYPE)
    if key in _PROGRAMS:
        return _PROGRAMS[key]

    import contextlib

    import concourse.bacc as bacc
    import concourse.mybir as mybir
    import concourse.tile as tile
    from concourse.masks import make_identity

    f32 = mybir.dt.float32
    bf16 = mybir.dt.bfloat16

    nc = bacc.Bacc("TRN2", target_bir_lowering=False, debug=False)

    xdt = mybir.dt.float8e4 if PROJ_DTYPE == "fp8" else bf16
    xT_d = nc.dram_tensor("xT", [E, S], xdt, kind="ExternalInput")
    w_d = {
        "q": nc.dram_tensor("wqT", [E, D], xdt, kind="ExternalInput"),
        "k": nc.dram_tensor("wkT", [E, D], xdt, kind="ExternalInput"),
        "v": nc.dram_tensor("wvT", [E, D], xdt, kind="ExternalInput"),
    }
    b_d = {
        "q": nc.dram_tensor("bq", [D, 1], f32, kind="ExternalInput"),
        "k": nc.dram_tensor("bk", [D, 1], f32, kind="ExternalInput"),
        "v": nc.dram_tensor("bv", [D, 1], f32, kind="ExternalInput"),
    }
    out_d = nc.dram_tensor("out", [S, D], f32, kind="ExternalOutput")

    with tile.TileContext(nc) as tc:
        with (
            tc.tile_pool(name="const", bufs=1) as cpool,
            tc.tile_pool(name="xt", bufs=5) as xpool,
            tc.tile_pool(name="qkv", bufs=1) as qkvpool,
            tc.tile_pool(name="probs", bufs=20) as ppool,
            tc.tile_pool(name="osb", bufs=2) as opool,
            tc.tile_pool(name="misc", bufs=2) as mpool,
            tc.tile_pool(name="proj_ps", bufs=3, space="PSUM") as proj_ps,
            tc.tile_pool(name="sc_ps", bufs=2, space="PSUM") as sc_ps,
            tc.tile_pool(name="vt_ps", bufs=1, space="PSUM") as vt_ps,
            tc.tile_pool(name="out_ps", bufs=2, space="PSUM") as out_ps,
        ):
            # ---- iteration-invariant setup ----
            ident = cpool.tile([P, P], bf16, tag="ident")
            make_identity(nc, ident[:])
            # cmaskT[k_local, q_local]: 0 where q >= k (valid), -1e30 where q < k
            cmaskT = cpool.tile([P, P], f32, tag="cmaskT")
            nc.gpsimd.memset(cmaskT[:], 0.0)
            nc.gpsimd.affine_select(
                out=cmaskT[:],
                in_=cmaskT[:],
                compare_op=mybir.AluOpType.is_ge,
                fill=NEG,
                base=0,
                # iota[r, c] = c - r ; keep (0.0) where c - r >= 0
                pattern=[[1, P]],
                channel_multiplier=-1,
            )

            w_sb = {}
            b_sb = {}
            for pj in ("q", "k", "v"):
                w_sb[pj] = cpool.tile(
                    [P, NE * D], xdt, name=f"w{pj}", tag=f"w{pj}"
                )
                nc.sync.dma_start(
                    w_sb[pj][:].rearrange("p (ec d) -> p ec d", ec=NE),
                    w_d[pj].rearrange("(ec p) d -> p ec d", p=P),
                )
                b_sb[pj] = cpool.tile([P, 1], f32, name=f"b{pj}", tag=f"b{pj}")
                nc.sync.dma_start(b_sb[pj][:], b_d[pj][:, :])

            if iters > 1:
                loop_cm = tc.For_i(
                    0,
                    iters,
                    1,
                    hint_engines=(
                        mybir.EngineType.PE,
                        mybir.EngineType.Activation,
                        mybir.EngineType.DVE,
                        mybir.EngineType.SP,
                        mybir.EngineType.Pool,
                    ),
                )
            else:
                loop_cm = contextlib.nullcontext()
            with loop_cm:
                _emit_body(
                    nc,
                    mybir,
                    pools={
                        "xpool": xpool,
                        "qkvpool": qkvpool,
                        "ppool": ppool,
                        "opool": opool,
                        "mpool": mpool,
                        "proj_ps": proj_ps,
                        "sc_ps": sc_ps,
                        "vt_ps": vt_ps,
                        "out_ps": out_ps,
                    },
                    dram={"xT": xT_d, "out": out_d},
                    consts={
                        "ident": ident,
                        "cmaskT": cmaskT,
                        "w": w_sb,
                        "b": b_sb,
                    },
                )

    nc.compile()
    _PROGRAMS[key] = nc
    return nc


def _emit_body(nc, mybir, pools, dram, consts):
    f32 = mybir.dt.float32
    bf16 = mybir.dt.bfloat16
    xdt = mybir.dt.float8e4 if PROJ_DTYPE == "fp8" else bf16
    xpool = pools["xpool"]
    qkvpool = pools["qkvpool"]
    ppool = pools["ppool"]
    opool = pools["opool"]
    mpool = pools["mpool"]
    proj_ps = pools["proj_ps"]
    sc_ps = pools["sc_ps"]
    vt_ps = pools["vt_ps"]
    out_ps = pools["out_ps"]
    xT_d = dram["xT"]
    out_d = dram["out"]
    ident = consts["ident"]
    cmaskT = consts["cmaskT"]
    w_sb = consts["w"]
    b_sb = consts["b"]

    # ---- xT loads: one tile per s-tile, prefetchable across iterations ----
    xT_v = xT_d.rearrange("(ec p) s -> p ec s", p=P)
    xt_st = []
    for st in range(NST):
        xt = xpool.tile([P, NE * ST], xdt, name=f"xt{st}", tag="xt")
        if st == 0:
            # fine-grained pieces: the first projection chain can start
            # after ~1 piece instead of waiting for the full 2 MB tile
            for ec in range(NE):
                nc.sync.dma_start(
                    xt[:, ec * ST : (ec + 1) * ST],
                    xT_v[:, ec, st * ST : (st + 1) * ST],
                )
        else:
            nc.sync.dma_start(
                xt[:].rearrange("p (ec s) -> p ec s", ec=NE),
                xT_v[:, :, st * ST : (st + 1) * ST],
            )
        xt_st.append(xt)

    # HAM warmup: keep the PE busy while the first xt pieces land so the
    # clock gate reaches 8/8 before the real matmul stream begins
    wps = pools["proj_ps"].tile([P, ST], mybir.dt.float32, name="warm", tag="proj")
    for wi in range(40):
        nc.tensor.matmul(
            wps[:, 0:P],
            lhsT=ident[:],
            rhs=ident[:],
            start=(wi == 0),
            stop=(wi == 39),
        )

    qT_sb = qkvpool.tile([P, S], bf16, tag="qT")
    kT_sb = qkvpool.tile([P, S], bf16, tag="kT")
    vT_sb = qkvpool.tile([P, S], bf16, tag="vT")
    v_sb = qkvpool.tile([P, NS * VW], bf16, tag="v")
    dest = {"q": qT_sb, "k": kT_sb, "v": vT_sb}

    # ones column of v_aug
    for sb in range(NS):
        nc.vector.memset(v_sb[:, sb * VW + D : (sb + 1) * VW], 1.0)

    probs_pieces = {}

    for st in range(NST):
        # ---- projections for this s-tile ----
        for pj in ("q", "k", "v") if "proj" in PHASES else ():
            ps = proj_ps.tile([P, ST], f32, tag="proj")
            if PROJ_DTYPE == "fp8":
                for g in range(NE // 2):
                    nc.tensor.matmul(
                        ps[:],
                        lhsT=w_sb[pj][:, 2 * g * D : (2 * g + 2) * D].rearrange(
                            "p (i d) -> p i d", i=2
                        ),
                        rhs=xt_st[st][:, 2 * g * ST : (2 * g + 2) * ST].rearrange(
                            "p (i s) -> p i s", i=2
                        ),
                        start=(g == 0),
                        stop=(g == NE // 2 - 1),
                        perf_mode=mybir.MatmulPerfMode.DoubleRow,
                    )
                nc.vector.tensor_scalar(
                    dest[pj][:, st * ST : (st + 1) * ST],
                    ps[:],
                    1.0 / W_SCALE,
                    b_sb[pj][:, 0:1],
                    op0=mybir.AluOpType.mult,
                    op1=mybir.AluOpType.add,
                )
            else:
                for ec in range(NE):
                    nc.tensor.matmul(
                        ps[:],
                        lhsT=w_sb[pj][:, ec * D : (ec + 1) * D],
                        rhs=xt_st[st][:, ec * ST : (ec + 1) * ST],
                        start=(ec == 0),
                        stop=(ec == NE - 1),
                    )
                nc.vector.tensor_scalar_add(
                    dest[pj][:, st * ST : (st + 1) * ST],
                    ps[:],
                    b_sb[pj][:, 0:1],
                )

        # ---- v natural blocks (PE transpose of vT) ----
        for sb in (range(st * (ST // P), (st + 1) * (ST // P)) if "vtrans" in PHASES else ()):
            tp = vt_ps.tile([P, P], bf16, tag="vt")
            nc.tensor.transpose(tp[:], vT_sb[:, sb * P : (sb + 1) * P], ident[:])
            nc.vector.tensor_copy(v_sb[:, sb * VW : sb * VW + D], tp[:])

        # ---- scoresT + exp for q-chunk c = st ----
        c = st
        for j in range(4 * c + 4) if "scores" in PHASES else ():
            qs = max(c * ST, j * P)
            w = (c + 1) * ST - qs
            sps = sc_ps.tile([P, ST], f32, tag="sc")
            nc.tensor.matmul(
                sps[:, :w],
                lhsT=kT_sb[:, j * P : (j + 1) * P],
                rhs=qT_sb[:, qs : qs + w],
                start=True,
                stop=True,
            )
            if j * P >= c * ST:
                # diagonal block occupies the first 128 columns
                nc.vector.tensor_add(sps[:, 0:P], sps[:, 0:P], cmaskT[:])
            prb = ppool.tile([P, ST], bf16, tag="probs")
            nc.scalar.activation(
                prb[:, :w],
                sps[:, :w],
                func=mybir.ActivationFunctionType.Exp,
                bias=0.0,
                scale=SCALE,
            )
            probs_pieces[(j, c)] = (prb, qs)

        # ---- AV + normalize + store for the 4 q-blocks of chunk c ----
        for i in range(4 * c, 4 * c + 4) if "av" in PHASES else ():
            ops = out_ps.tile([P, VW], f32, tag="out")
            for j in range(i + 1):
                prb, qs = probs_pieces[(j, c)]
                off = i * P - qs
                nc.tensor.matmul(
                    ops[:],
                    lhsT=prb[:, off : off + P],
                    rhs=v_sb[:, j * VW : (j + 1) * VW],
                    start=(j == 0),
                    stop=(j == i),
                )
            recip = mpool.tile([P, 1], f32, tag="recip")
            nc.vector.reciprocal(recip[:], ops[:, D : D + 1])
            osb = opool.tile([P, D], f32, tag="osb")
            nc.vector.tensor_scalar_mul(osb[:], ops[:, 0:D], recip[:, 0:1])
            nc.sync.dma_start(out_d[i * P : (i + 1) * P, :], osb[:])


def make_in_maps(x, Wq, bq, Wk, bk, Wv, bv):
    import ml_dtypes

    if PROJ_DTYPE == "fp8":
        xdt = ml_dtypes.float8_e4m3
        wscale = W_SCALE
    else:
        xdt = ml_dtypes.bfloat16
        wscale = 1.0
    x = np.asarray(x, dtype=np.float32)

    def wcast(W):
        return np.ascontiguousarray(
            np.asarray(W, dtype=np.float32).T * wscale
        ).astype(xdt)

    shared = {
        "wqT": wcast(Wq),
        "wkT": wcast(Wk),
        "wvT": wcast(Wv),
        "bq": np.asarray(bq, dtype=np.float32).reshape(D, 1).copy(),
        "bk": np.asarray(bk, dtype=np.float32).reshape(D, 1).copy(),
        "bv": np.asarray(bv, dtype=np.float32).reshape(D, 1).copy(),
    }
    return [
        {"xT": np.ascontiguousarray(x[b].T).astype(xdt), **shared}
        for b in range(B)
    ]


def kernel(x, Wq, bq, Wk, bk, Wv, bv):
    from concourse.bass_utils import run_bass_kernel_spmd

    nc = build_program()
    in_maps = make_in_maps(x, Wq, bq, Wk, bk, Wv, bv)
    res = run_bass_kernel_spmd(nc, in_maps, list(range(B)))
    return np.stack([res.results[i]["out"] for i in range(B)], axis=0)


# revision 13
# speedup vs baseline: 1.0481x; 1.0481x over previous
"""Single-head causal attention (B=8, S=2048, E=2048, D=128) on 8 trn2 cores.

Sharding: data-parallel over batch — one batch element per NeuronCore.

Host marshaling per core: xT = x[b].T and WT = W.T, both cast to bf16
(the kernel computes in bf16 with f32 accumulation; casting during input
marshaling is numerically identical to casting on device).

Per-core dataflow (bf16 matmuls, f32 PSUM accumulation):
  - projections produce qT/kT/vT in [D, S] layout; bias added during the
    VectorE PSUM->SBUF evacuation (per-partition scalar add)
  - vT is re-transposed on the PE into natural [S, D] blocks, augmented
    with a ones column (col 128): the AV matmul then yields the softmax
    denominator for free as output column 128
  - scoresT[k, q] per k-block j: single matmul (K=D=128), exact causal
    trim of the q range; diagonal 128-block masked by adding -1e30;
    ScalarE computes exp(scale*s) straight out of PSUM into bf16 probsT
  - AV per q-block i accumulates probsT_j.T @ v_aug_j over j<=i in PSUM;
    VectorE takes 1/denominator and applies it during the final
    evacuation; DMA stores the natural-layout [128, 128] f32 result
"""

import math

import numpy as np

B = 8
S = 2048
E = 2048
D = 128
P = 128
NE = E // P  # 16 contraction chunks
NS = S // P  # 16 sequence blocks
ST = 512  # s-tile width for projections / score chunks
NST = S // ST  # 4
VW = D + 1  # v block width incl. ones column
SCALE = 1.0 / math.sqrt(S)
NEG = -1.0e30

_PROGRAMS = {}

# which phases to emit (for microbenchmarking): subset of
# {"proj", "vtrans", "scores", "av", "store"}
PHASES = frozenset({"proj", "vtrans", "scores", "av", "store"})

# projection matmul precision: "fp8" (DoubleRow, 2x PE) or "bf16"
PROJ_DTYPE = "bf16"
W_SCALE = 256.0  # host pre-scale of W before fp8 quantization


def build_program(iters=1):
    global _PROGRAMS
    key = (iters, PHASES, PROJ_DTYPE)
    if key in _PROGRAMS:
        return _PROGRAMS[key]

    import contextlib

    import concourse.bacc as bacc
    import concourse.mybir as mybir
    import concourse.tile as tile
    from concourse.masks import make_identity

    f32 = mybir.dt.float32
    bf16 = mybir.dt.bfloat16

    nc = bacc.Bacc("TRN2", target_bir_lowering=False, debug=False)

    xdt = mybir.dt.float8e4 if PROJ_DTYPE == "fp8" else bf16
    xT_d = nc.dram_tensor("xT", [E, S], xdt, kind="ExternalInput")
    w_d = {
        "q": nc.dram_tensor("wqT", [E, D], xdt, kind="ExternalInput"),
        "k": nc.dram_tensor("wkT", [E, D], xdt, kind="ExternalInput"),
        "v": nc.dram_tensor("wvT", [E, D], xdt, kind="ExternalInput"),
    }
    b_d = {
        "q": nc.dram_tensor("bq", [D, 1], f32, kind="ExternalInput"),
        "k": nc.dram_tensor("bk", [D, 1], f32, kind="ExternalInput"),
        "v": nc.dram_tensor("bv", [D, 1], f32, kind="ExternalInput"),
    }
    out_d = nc.dram_tensor("out", [S, D], f32, kind="ExternalOutput")

    with tile.TileContext(nc) as tc:
        with (
            tc.tile_pool(name="const", bufs=1) as cpool,
            tc.tile_pool(name="xt", bufs=5) as xpool,
            tc.tile_pool(name="qkv", bufs=1) as qkvpool,
            tc.tile_pool(name="probs", bufs=20) as ppool,
            tc.tile_pool(name="osb", bufs=2) as opool,
            tc.tile_pool(name="misc", bufs=2) as mpool,
            tc.tile_pool(name="proj_ps", bufs=3, space="PSUM") as proj_ps,
            tc.tile_pool(name="sc_ps", bufs=2, space="PSUM") as sc_ps,
            tc.tile_pool(name="vt_ps", bufs=1, space="PSUM") as vt_ps,
            tc.tile_pool(name="out_ps", bufs=2, space="PSUM") as out_ps,
        ):
            # ---- iteration-invariant setup ----
            ident = cpool.tile([P, P], bf16, tag="ident")
            make_identity(nc, ident[:])
            # cmaskT[k_local, q_local]: 0 where q >= k (valid), -1e30 where q < k
            cmaskT = cpool.tile([P, P], f32, tag="cmaskT")
            nc.gpsimd.memset(cmaskT[:], 0.0)
            nc.gpsimd.affine_select(
                out=cmaskT[:],
                in_=cmaskT[:],
                compare_op=mybir.AluOpType.is_ge,
                fill=NEG,
                base=0,
                # iota[r, c] = c - r ; keep (0.0) where c - r >= 0
                pattern=[[1, P]],
                channel_multiplier=-1,
            )

            w_sb = {}
            b_sb = {}
            for pj in ("q", "k", "v"):
                w_sb[pj] = cpool.tile(
                    [P, NE * D], xdt, name=f"w{pj}", tag=f"w{pj}"
                )
                nc.sync.dma_start(
                    w_sb[pj][:].rearrange("p (ec d) -> p ec d", ec=NE),
                    w_d[pj].rearrange("(ec p) d -> p ec d", p=P),
                )
                b_sb[pj] = cpool.tile([P, 1], f32, name=f"b{pj}", tag=f"b{pj}")
                nc.sync.dma_start(b_sb[pj][:], b_d[pj][:, :])

            # HAM warmup while the first DMAs land (runs once, cold)
            wps = proj_ps.tile([P, ST], f32, name="warm", tag="proj")
            for wi in range(40):
                nc.tensor.matmul(
                    wps[:, 0:P],
                    lhsT=ident[:],
                    rhs=ident[:],
                    start=(wi == 0),
                    stop=(wi == 39),
                )

            if iters > 1:
                loop_cm = tc.For_i(
                    0,
                    iters,
                    1,
                    hint_engines=(
                        mybir.EngineType.PE,
                        mybir.EngineType.Activation,
                        mybir.EngineType.DVE,
                        mybir.EngineType.SP,
                        mybir.EngineType.Pool,
                    ),
                )
            else:
                loop_cm = contextlib.nullcontext()
            with loop_cm:
                _emit_body(
                    nc,
                    mybir,
                    pools={
                        "xpool": xpool,
                        "qkvpool": qkvpool,
                        "ppool": ppool,
                        "opool": opool,
                        "mpool": mpool,
                        "proj_ps": proj_ps,
                        "sc_ps": sc_ps,
                        "vt_ps": vt_ps,
                        "out_ps": out_ps,
                    },
                    dram={"xT": xT_d, "out": out_d},
                    consts={
                        "ident": ident,
                        "cmaskT": cmaskT,
                        "w": w_sb,
                        "b": b_sb,
                    },
                )

    nc.compile()
    _PROGRAMS[key] = nc
    return nc


def _emit_body(nc, mybir, pools, dram, consts):
    f32 = mybir.dt.float32
    bf16 = mybir.dt.bfloat16
    xdt = mybir.dt.float8e4 if PROJ_DTYPE == "fp8" else bf16
    xpool = pools["xpool"]
    qkvpool = pools["qkvpool"]
    ppool = pools["ppool"]
    opool = pools["opool"]
    mpool = pools["mpool"]
    proj_ps = pools["proj_ps"]
    sc_ps = pools["sc_ps"]
    vt_ps = pools["vt_ps"]
    out_ps = pools["out_ps"]
    xT_d = dram["xT"]
    out_d = dram["out"]
    ident = consts["ident"]
    cmaskT = consts["cmaskT"]
    w_sb = consts["w"]
    b_sb = consts["b"]

    # ---- xT loads: one tile per s-tile, prefetchable across iterations ----
    xT_v = xT_d.rearrange("(ec p) s -> p ec s", p=P)
    xt_st = []
    for st in range(NST):
        xt = xpool.tile([P, NE * ST], xdt, name=f"xt{st}", tag="xt")
        if st == 0:
            # fine-grained pieces: the first projection chain can start
            # after ~1 piece instead of waiting for the full 2 MB tile
            for ec in range(NE):
                nc.sync.dma_start(
                    xt[:, ec * ST : (ec + 1) * ST],
                    xT_v[:, ec, st * ST : (st + 1) * ST],
                )
        else:
            nc.sync.dma_start(
                xt[:].rearrange("p (ec s) -> p ec s", ec=NE),
                xT_v[:, :, st * ST : (st + 1) * ST],
            )
        xt_st.append(xt)

    qT_sb = qkvpool.tile([P, S], bf16, tag="qT")
    kT_sb = qkvpool.tile([P, S], bf16, tag="kT")
    vT_sb = qkvpool.tile([P, S], bf16, tag="vT")
    v_sb = qkvpool.tile([P, NS * VW], bf16, tag="v")
    dest = {"q": qT_sb, "k": kT_sb, "v": vT_sb}

    # ones column of v_aug
    for sb in range(NS):
        nc.vector.memset(v_sb[:, sb * VW + D : (sb + 1) * VW], 1.0)

    probs_pieces = {}

    for st in range(NST):
        # ---- projections for this s-tile ----
        for pj in ("q", "k", "v") if "proj" in PHASES else ():
            ps = proj_ps.tile([P, ST], f32, tag="proj")
            if PROJ_DTYPE == "fp8":
                for g in range(NE // 2):
                    nc.tensor.matmul(
                        ps[:],
                        lhsT=w_sb[pj][:, 2 * g * D : (2 * g + 2) * D].rearrange(
                            "p (i d) -> p i d", i=2
                        ),
                        rhs=xt_st[st][:, 2 * g * ST : (2 * g + 2) * ST].rearrange(
                            "p (i s) -> p i s", i=2
                        ),
                        start=(g == 0),
                        stop=(g == NE // 2 - 1),
                        perf_mode=mybir.MatmulPerfMode.DoubleRow,
                    )
                nc.vector.tensor_scalar(
                    dest[pj][:, st * ST : (st + 1) * ST],
                    ps[:],
                    1.0 / W_SCALE,
                    b_sb[pj][:, 0:1],
                    op0=mybir.AluOpType.mult,
                    op1=mybir.AluOpType.add,
                )
            else:
                for ec in range(NE):
                    nc.tensor.matmul(
                        ps[:],
                        lhsT=w_sb[pj][:, ec * D : (ec + 1) * D],
                        rhs=xt_st[st][:, ec * ST : (ec + 1) * ST],
                        start=(ec == 0),
                        stop=(ec == NE - 1),
                    )
                nc.vector.tensor_scalar_add(
                    dest[pj][:, st * ST : (st + 1) * ST],
                    ps[:],
                    b_sb[pj][:, 0:1],
                )

        # ---- v natural blocks (PE transpose of vT) ----
        for sb in (range(st * (ST // P), (st + 1) * (ST // P)) if "vtrans" in PHASES else ()):
            tp = vt_ps.tile([P, P], bf16, tag="vt")
            nc.tensor.transpose(tp[:], vT_sb[:, sb * P : (sb + 1) * P], ident[:])
            nc.vector.tensor_copy(v_sb[:, sb * VW : sb * VW + D], tp[:])

        # ---- scoresT + exp for q-chunk c = st ----
        c = st
        for j in range(4 * c + 4) if "scores" in PHASES else ():
            qs = max(c * ST, j * P)
            w = (c + 1) * ST - qs
            sps = sc_ps.tile([P, ST], f32, tag="sc")
            nc.tensor.matmul(
                sps[:, :w],
                lhsT=kT_sb[:, j * P : (j + 1) * P],
                rhs=qT_sb[:, qs : qs + w],
                start=True,
                stop=True,
            )
            if j * P >= c * ST:
                # diagonal block occupies the first 128 columns
                nc.vector.tensor_add(sps[:, 0:P], sps[:, 0:P], cmaskT[:])
            prb = ppool.tile([P, ST], bf16, tag="probs")
            nc.scalar.activation(
                prb[:, :w],
                sps[:, :w],
                func=mybir.ActivationFunctionType.Exp,
                bias=0.0,
                scale=SCALE,
            )
            probs_pieces[(j, c)] = (prb, qs)

        # ---- AV + normalize + store for the 4 q-blocks of chunk c ----
        for i in range(4 * c, 4 * c + 4) if "av" in PHASES else ():
            ops = out_ps.tile([P, VW], f32, tag="out")
            for j in range(i + 1):
                prb, qs = probs_pieces[(j, c)]
                off = i * P - qs
                nc.tensor.matmul(
                    ops[:],
                    lhsT=prb[:, off : off + P],
                    rhs=v_sb[:, j * VW : (j + 1) * VW],
                    start=(j == 0),
                    stop=(j == i),
                )
            recip = mpool.tile([P, 1], f32, tag="recip")
            nc.vector.reciprocal(recip[:], ops[:, D : D + 1])
            osb = opool.tile([P, D], f32, tag="osb")
            nc.vector.tensor_scalar_mul(osb[:], ops[:, 0:D], recip[:, 0:1])
            nc.sync.dma_start(out_d[i * P : (i + 1) * P, :], osb[:])


def make_in_maps(x, Wq, bq, Wk, bk, Wv, bv):
    import ml_dtypes

    if PROJ_DTYPE == "fp8":
        xdt = ml_dtypes.float8_e4m3
        wscale = W_SCALE
    else:
        xdt = ml_dtypes.bfloat16
        wscale = 1.0
    x = np.asarray(x, dtype=np.float32)

    def wcast(W):
        return np.ascontiguousarray(
            np.asarray(W, dtype=np.float32).T * wscale
        ).astype(xdt)

    shared = {
        "wqT": wcast(Wq),
        "wkT": wcast(Wk),
        "wvT": wcast(Wv),
        "bq": np.asarray(bq, dtype=np.float32).reshape(D, 1).copy(),
        "bk": np.asarray(bk, dtype=np.float32).reshape(D, 1).copy(),
        "bv": np.asarray(bv, dtype=np.float32).reshape(D, 1).copy(),
    }
    return [
        {"xT": np.ascontiguousarray(x[b].T).astype(xdt), **shared}
        for b in range(B)
    ]


def kernel(x, Wq, bq, Wk, bk, Wv, bv):
    from concourse.bass_utils import run_bass_kernel_spmd

    nc = build_program()
    in_maps = make_in_maps(x, Wq, bq, Wk, bk, Wv, bv)
    res = run_bass_kernel_spmd(nc, in_maps, list(range(B)))
    return np.stack([res.results[i]["out"] for i in range(B)], axis=0)
